# revision 12
# baseline (speedup 1.0000x reference)
"""Multi-head attention (B=4, N=2048, DM=1024, H=16, DH=64) on 8 trn2 cores.

Sharding: core c -> (batch b = c//2, head-group hg = c%2 of 8 heads).
Each core computes qkv for its 8 heads, masked softmax attention, and a
partial output projection over its 512 head-dims.  Host sums the two
partials per batch and adds the bias.

Device-side layout ("feature-major"):
  - x^T [DM, N] so QK projection emits q^T/k^T [64, N] per head directly.
  - mask folded into q^T (x SCALE*m_i) and k^T (x m_j): masked score
    pairs become exp(0)=1; a rank-1 correction matmul (-m_i * C_h, with
    C_h = sum_{masked j} v_h[j,:]) cancels them for live queries, and
    dead queries (m_i=0) fall out as the exact uniform-softmax rows the
    reference produces.
  - v stored token-major with an appended ones column per head, so the
    PV matmul accumulates the softmax denominator for free.
"""

import sys

sys.path.insert(0, "/opt/trn_rl_repo")

import numpy as np
import ml_dtypes

B, N, DM, H, DH = 4, 2048, 1024, 16, 64
SCALE = DH**-0.5
NCORES = 8
HG = 2  # head groups (tensor-parallel factor)
HL = H // HG  # 8 heads per core
FQK = HL * 2 * DH  # 1024 qk features per core
FV = HL * DH  # 512 v features per core
P = 128
NT = N // P  # 16 token tiles
DMT = DM // P  # 8 dm tiles
VW = DH + 1  # 65: v columns + ones column
VROW = HL * VW  # 520
HT = FV // P  # 4 head-dim tiles for the projection

_CACHE = {}


def _build_program():
    import concourse.bass as bass
    import concourse.mybir as mybir
    import concourse.tile as tile
    from concourse import bacc
    from concourse.bass import ts
    from concourse.masks import make_identity

    bf = mybir.dt.bfloat16
    f32 = mybir.dt.float32
    EXP = mybir.ActivationFunctionType.Exp

    nc = bacc.Bacc(
        "TRN2", target_bir_lowering=False, debug=False, num_devices=NCORES
    )
    xT = nc.dram_tensor("xT", [DM, N], bf, kind="ExternalInput")
    wqk = nc.dram_tensor("wqk", [DM, FQK], bf, kind="ExternalInput")
    wv = nc.dram_tensor("wv", [DM, FV], bf, kind="ExternalInput")
    wout = nc.dram_tensor("wout", [FV, DM], bf, kind="ExternalInput")
    qkmask = nc.dram_tensor("qkmask", [P, N], f32, kind="ExternalInput")
    mrow = nc.dram_tensor("mrow", [1, N], bf, kind="ExternalInput")
    iminv = nc.dram_tensor("iminv", [P, NT], bf, kind="ExternalInput")
    out = nc.dram_tensor("out", [N, DM], f32, kind="ExternalOutput")

    with tile.TileContext(nc) as tc:
        with tc.tile_pool(name="const", bufs=1) as cp:
            xT_sb = cp.tile([P, DMT * N], bf, tag="xT")
            wqk_sb = cp.tile([P, DMT * FQK], bf, tag="wqk")
            wv_sb = cp.tile([P, DMT * FV], bf, tag="wv")
            wout_sb = cp.tile([P, HT * DM], bf, tag="wout")
            qkm_sb = cp.tile([P, N], f32, tag="qkm")
            mrow_sb = cp.tile([1, N], bf, tag="mrow")
            iminv_sb = cp.tile([P, NT], bf, tag="iminv")
            ident = cp.tile([P, P], bf, tag="ident")
            vplus = cp.tile([P, NT * VROW], bf, tag="vplus")
            qk_all = cp.tile([P, HL * N], bf, tag="qkall")
            attT = cp.tile([P, HT * N], bf, tag="attT")
            att_pair = cp.tile([P, NT * P], bf, tag="attpair")
            c_sb = cp.tile([1, VROW], bf, tag="csb")

            for dmt in range(DMT):
                nc.sync.dma_start(out=xT_sb[:, ts(dmt, N)], in_=xT[ts(dmt, P), :])
                nc.sync.dma_start(out=wqk_sb[:, ts(dmt, FQK)], in_=wqk[ts(dmt, P), :])
                nc.sync.dma_start(out=wv_sb[:, ts(dmt, FV)], in_=wv[ts(dmt, P), :])
            for ht in range(HT):
                nc.sync.dma_start(out=wout_sb[:, ts(ht, DM)], in_=wout[ts(ht, P), :])
            nc.sync.dma_start(out=qkm_sb[:, :], in_=qkmask[:, :])
            nc.sync.dma_start(out=mrow_sb[:, :], in_=mrow[:, :])
            nc.sync.dma_start(out=iminv_sb[:, :], in_=iminv[:, :])
            make_identity(nc, ident)

            vp4 = vplus.rearrange("p (t g c) -> p t g c", t=NT, g=HL, c=VW)
            nc.gpsimd.memset(vp4[:, :, :, DH], 1.0)

            # Prime the DVE vector clock on the mask DMA so the first
            # tensor_mul needs only the PE wait (the 3-src DVE TT
            # instruction encodes a single sync-wait slot).
            scratch = cp.tile([1, 1], f32, tag="scratch")
            nc.vector.tensor_copy(scratch, qkm_sb[0:1, 0:1])

            # ---- stage 1: QK projection, mask+scale fold, cast to bf16 ----
            # wqk feature layout: cols 0:512 = q features (4 f-tiles of 2
            # heads), cols 512:1024 = k features.  Head h's q lives in
            # f-tile h//2 at partition (h%2)*64, same for k in f-tile
            # 4 + h//2 — so the scores matmul operands share a base
            # partition, which the PE requires.
            with tc.tile_pool(name="psqk", bufs=1, space="PSUM") as pqk:
                for ft in range(HL):
                    ps_qk = pqk.tile([P, N], f32, tag="qk")
                    for dmt in range(DMT):
                        lhsT = wqk_sb[:, dmt * FQK + ft * P : dmt * FQK + (ft + 1) * P]
                        for ch in range(4):
                            nc.tensor.matmul(
                                ps_qk[:, ts(ch, 512)],
                                lhsT,
                                xT_sb[:, dmt * N + ch * 512 : dmt * N + (ch + 1) * 512],
                                start=(dmt == 0),
                                stop=(dmt == DMT - 1),
                            )
                    # SCALE is pre-baked into the q-projection weights on
                    # the host, so q and k tiles both just multiply by mask.
                    nc.vector.tensor_mul(
                        qk_all[:, ts(ft, N)], ps_qk[:, :], qkm_sb[:, :]
                    )

            # ---- stage 2: V projection (token-major) + masked-v row C ----
            with (
                tc.tile_pool(name="psv", bufs=2, space="PSUM") as pv,
                tc.tile_pool(name="psc", bufs=1, space="PSUM") as pc,
            ):
                for tt in range(NT):
                    ps_v = pv.tile([P, FV], f32, tag="v")
                    for dmt in range(DMT):
                        nc.tensor.matmul(
                            ps_v[:, :],
                            xT_sb[:, dmt * N + tt * P : dmt * N + (tt + 1) * P],
                            wv_sb[:, ts(dmt, FV)],
                            start=(dmt == 0),
                            stop=(dmt == DMT - 1),
                        )
                    nc.vector.tensor_copy(
                        vp4[:, tt, :, 0:DH],
                        ps_v.rearrange("p (g c) -> p g c", g=HL, c=DH),
                    )
                ps_c0 = pc.tile([1, VROW // 2], f32, tag="c0")
                ps_c1 = pc.tile([1, VROW // 2], f32, tag="c1")
                for jt in range(NT):
                    nc.tensor.matmul(
                        ps_c0[:, :],
                        iminv_sb[:, jt : jt + 1],
                        vplus[:, jt * VROW : jt * VROW + VROW // 2],
                        start=(jt == 0),
                        stop=(jt == NT - 1),
                    )
                    nc.tensor.matmul(
                        ps_c1[:, :],
                        iminv_sb[:, jt : jt + 1],
                        vplus[:, jt * VROW + VROW // 2 : (jt + 1) * VROW],
                        start=(jt == 0),
                        stop=(jt == NT - 1),
                    )
                nc.vector.tensor_scalar_mul(c_sb[:, 0 : VROW // 2], ps_c0[:, :], -1.0)
                nc.vector.tensor_scalar_mul(c_sb[:, VROW // 2 : VROW], ps_c1[:, :], -1.0)

            # ---- stage 3: per-head scores^T -> exp -> PV -> normalize ----
            with (
                tc.tile_pool(name="pss", bufs=2, space="PSUM") as pss,
                tc.tile_pool(name="psa", bufs=1, space="PSUM") as psa,
                tc.tile_pool(name="tpool", bufs=2) as tp,
                tc.tile_pool(name="spool", bufs=2) as sp,
            ):
                for h in range(HL):
                    p0 = (h % 2) * 64  # base partition of this head's q/k
                    qcol = (h // 2) * N
                    kcol = (HL // 2 + h // 2) * N
                    pa = psa.tile([P, NT * P], f32, tag="att", name="pa")
                    for jt in range(NT):
                        t_sb = tp.tile([P, N], bf, tag="t", name="t_sb")
                        kT = qk_all[p0 : p0 + 64, kcol + jt * P : kcol + (jt + 1) * P]
                        for half in range(2):
                            ps_s = pss.tile([P, 1024], f32, tag="s", name="ps_s")
                            for ch in range(2):
                                c0 = qcol + half * 1024 + ch * 512
                                nc.tensor.matmul(
                                    ps_s[:, ts(ch, 512)],
                                    kT,
                                    qk_all[p0 : p0 + 64, c0 : c0 + 512],
                                    start=True,
                                    stop=True,
                                )
                            nc.scalar.activation(
                                t_sb[:, ts(half, 1024)], ps_s[:, :], EXP
                            )
                        vslice = vplus[:, jt * VROW + h * VW : jt * VROW + (h + 1) * VW]
                        # One accumulation group per PSUM bank (4 it-regions
                        # of 512B each): only the bank's first matmul starts
                        # it; later regions overwrite via pending-zero.
                        for it in range(NT):
                            nc.tensor.matmul(
                                pa[:, it * P : it * P + VW],
                                t_sb[:, ts(it, P)],
                                vslice,
                                start=(jt == 0 and it % 4 == 0),
                                stop=False,
                            )
                    for it in range(NT):
                        nc.tensor.matmul(
                            pa[:, it * P : it * P + VW],
                            mrow_sb[:, ts(it, P)],
                            c_sb[:, h * VW : (h + 1) * VW],
                            start=False,
                            stop=(it % 4 == 3),
                        )
                    r_sb = sp.tile([P, NT], f32, tag="r", name="r_sb")
                    pa3 = pa.rearrange("p (t c) -> p t c", t=NT, c=P)
                    nc.vector.reciprocal(r_sb[:, :], pa3[:, :, DH])
                    for it in range(NT):
                        dst = att_pair[:, it * P + (h % 2) * DH : it * P + (h % 2) * DH + DH]
                        nc.vector.tensor_scalar_mul(
                            dst, pa[:, it * P : it * P + DH], r_sb[:, it : it + 1]
                        )
                    if h % 2 == 1:
                        for it in range(NT):
                            ps_tr = pss.tile([P, P], bf, tag="s", name="ps_tr")
                            nc.tensor.transpose(ps_tr[:, :], att_pair[:, ts(it, P)], ident)
                            nc.vector.tensor_copy(
                                attT[:, (h // 2) * N + it * P : (h // 2) * N + (it + 1) * P],
                                ps_tr[:, :],
                            )

            # ---- stage 4: partial output projection ----
            with (
                tc.tile_pool(name="pso", bufs=2, space="PSUM") as po,
                tc.tile_pool(name="opool", bufs=2) as op,
            ):
                for it in range(NT):
                    ps_o = po.tile([P, DM], f32, tag="o", name="ps_o")
                    for ht in range(HT):
                        lhsT = attT[:, ht * N + it * P : ht * N + (it + 1) * P]
                        for ch in range(2):
                            nc.tensor.matmul(
                                ps_o[:, ts(ch, 512)],
                                lhsT,
                                wout_sb[:, ht * DM + ch * 512 : ht * DM + (ch + 1) * 512],
                                start=(ht == 0),
                                stop=(ht == HT - 1),
                            )
                    o_sb = op.tile([P, DM], f32, tag="ob", name="o_sb")
                    nc.vector.tensor_copy(o_sb[:, :], ps_o[:, :])
                    nc.sync.dma_start(out=out[ts(it, P), :], in_=o_sb[:, :])

    nc.compile()
    return nc


def _shard_inputs(x, w_qkv, w_out, b_out, mask):
    """Build the per-core input maps (host-side sharding + layout prep)."""
    bf = ml_dtypes.bfloat16
    x = np.asarray(x, dtype=np.float32)
    w_qkv = np.asarray(w_qkv, dtype=np.float32)
    w_out = np.asarray(w_out, dtype=np.float32)
    mask = np.asarray(mask)

    # w_qkv columns: head h occupies cols [h*192, (h+1)*192) as q|k|v of 64.
    w3 = w_qkv.reshape(DM, H, 3, DH)
    in_maps = []
    for c in range(NCORES):
        b, hg = c // HG, c % HG
        heads = range(hg * HL, (hg + 1) * HL)
        # q features for all 8 heads (cols 0:512, pre-scaled by SCALE),
        # then k features
        wqk_c = np.ascontiguousarray(
            np.concatenate(
                [
                    w3[:, hg * HL : (hg + 1) * HL, 0, :].reshape(DM, FV) * SCALE,
                    w3[:, hg * HL : (hg + 1) * HL, 1, :].reshape(DM, FV),
                ],
                axis=1,
            )
        ).astype(bf)
        wv_c = np.ascontiguousarray(
            w3[:, hg * HL : (hg + 1) * HL, 2, :].reshape(DM, FV)
        ).astype(bf)
        wout_c = np.ascontiguousarray(
            w_out[hg * FV : (hg + 1) * FV, :]
        ).astype(bf)
        xT_c = np.ascontiguousarray(x[b].T).astype(bf)

        m = mask[b].astype(np.float32)  # [N] of 0/1
        qkm = np.broadcast_to(m[None, :], (P, N)).copy()
        mrow_c = m[None, :].astype(bf)
        iminv_c = np.ascontiguousarray(
            (1.0 - m).reshape(NT, P).T
        ).astype(bf)

        in_maps.append(
            {
                "xT": xT_c,
                "wqk": wqk_c,
                "wv": wv_c,
                "wout": wout_c,
                "qkmask": qkm,
                "mrow": mrow_c,
                "iminv": iminv_c,
            }
        )
    return in_maps


def kernel(x, w_qkv, w_out, b_out, mask):
    from concourse.bass_utils import run_bass_kernel_spmd

    if "nc" not in _CACHE:
        _CACHE["nc"] = _build_program()
    nc = _CACHE["nc"]

    in_maps = _shard_inputs(x, w_qkv, w_out, b_out, mask)
    res = run_bass_kernel_spmd(nc, in_maps, list(range(NCORES))).results

    b_out = np.asarray(b_out, dtype=np.float32)
    out = np.empty((B, N, DM), np.float32)
    for b in range(B):
        out[b] = res[HG * b]["out"] + res[HG * b + 1]["out"] + b_out[None, :]
    return out


# revision 15
# speedup vs baseline: 1.1170x; 1.1170x over previous
"""Multi-head attention (B=4, N=2048, DM=1024, H=16, DH=64) on 8 trn2 cores.

Sharding: core c -> (batch b = c//2, head-group hg = c%2 of 8 heads).
Each core computes qkv for its 8 heads, masked softmax attention, and a
partial output projection over its 512 head-dims.  Host sums the two
partials per batch and adds the bias.

Device-side layout ("feature-major"):
  - x^T [DM, N] so QK projection emits q^T/k^T [64, N] per head directly.
  - mask folded into q^T (x SCALE*m_i, SCALE pre-baked in w_q) and k^T
    (x m_j): masked score pairs become exp(0)=1; a rank-1 correction
    matmul (-m_i * C_h, with C_h = sum_{masked j} v_h[j,:]) cancels them
    for live queries, and dead queries (m_i=0) fall out as the exact
    uniform-softmax rows the reference produces.
  - v stored token-major with an appended ones column per head, so the
    PV matmul accumulates the softmax denominator for free.
  - per-head-pair QK projection is interleaved into the head loop so the
    PE keeps busy while ACT (exp) is the softmax bottleneck.
"""

import sys

sys.path.insert(0, "/opt/trn_rl_repo")

import numpy as np
import ml_dtypes

B, N, DM, H, DH = 4, 2048, 1024, 16, 64
SCALE = DH**-0.5
NCORES = 8
HG = 2  # head groups (tensor-parallel factor)
HL = H // HG  # 8 heads per core
NP = HL // 2  # 4 head pairs
FQK = HL * 2 * DH  # 1024 qk features per core
FV = HL * DH  # 512 v features per core
P = 128
NT = N // P  # 16 token tiles
DMT = DM // P  # 8 dm tiles
VW = DH + 1  # 65: v columns + ones column
VROW = HL * VW  # 520
HT = FV // P  # 4 head-dim tiles for the projection
NH = N // 2  # 1024: i-half width
NHT = NT // 2  # 8 token tiles per i-half

_CACHE = {}


def _build_program():
    import concourse.mybir as mybir
    import concourse.tile as tile
    from concourse import bacc
    from concourse.bass import ts
    from concourse.masks import make_identity

    bf = mybir.dt.bfloat16
    f32 = mybir.dt.float32
    EXP = mybir.ActivationFunctionType.Exp

    nc = bacc.Bacc(
        "TRN2", target_bir_lowering=False, debug=False, num_devices=NCORES
    )
    xT = nc.dram_tensor("xT", [DM, N], bf, kind="ExternalInput")
    wqk = nc.dram_tensor("wqk", [DM, FQK], bf, kind="ExternalInput")
    wv = nc.dram_tensor("wv", [DM, FV], bf, kind="ExternalInput")
    wout = nc.dram_tensor("wout", [FV, DM], bf, kind="ExternalInput")
    qkmask = nc.dram_tensor("qkmask", [P, N], f32, kind="ExternalInput")
    mrow = nc.dram_tensor("mrow", [1, N], bf, kind="ExternalInput")
    iminv = nc.dram_tensor("iminv", [P, NT], bf, kind="ExternalInput")
    out = nc.dram_tensor("out", [N, DM], f32, kind="ExternalOutput")

    with tile.TileContext(nc) as tc:
        with tc.tile_pool(name="const", bufs=1) as cp:
            xT_sb = cp.tile([P, DMT * N], bf, tag="xT")
            wqk_sb = cp.tile([P, DMT * FQK], bf, tag="wqk")
            wv_sb = cp.tile([P, DMT * FV], bf, tag="wv")
            wout_sb = cp.tile([P, HT * DM], bf, tag="wout")
            qkm_sb = cp.tile([P, N], f32, tag="qkm")
            mrow_sb = cp.tile([1, N], bf, tag="mrow")
            iminv_sb = cp.tile([P, NT], bf, tag="iminv")
            ident = cp.tile([P, P], bf, tag="ident")
            vplus = cp.tile([P, NT * VROW], bf, tag="vplus")
            qk_all = cp.tile([P, HL * N], bf, tag="qkall")
            attT = cp.tile([P, HT * N], bf, tag="attT")
            att_pair = cp.tile([P, NT * P], bf, tag="attpair")
            c_sb = cp.tile([1, VROW], bf, tag="csb")

            for dmt in range(DMT):
                nc.sync.dma_start(out=xT_sb[:, ts(dmt, N)], in_=xT[ts(dmt, P), :])
                nc.sync.dma_start(out=wqk_sb[:, ts(dmt, FQK)], in_=wqk[ts(dmt, P), :])
                nc.sync.dma_start(out=wv_sb[:, ts(dmt, FV)], in_=wv[ts(dmt, P), :])
            for ht in range(HT):
                nc.sync.dma_start(out=wout_sb[:, ts(ht, DM)], in_=wout[ts(ht, P), :])
            nc.sync.dma_start(out=qkm_sb[:, :], in_=qkmask[:, :])
            nc.sync.dma_start(out=mrow_sb[:, :], in_=mrow[:, :])
            nc.sync.dma_start(out=iminv_sb[:, :], in_=iminv[:, :])
            make_identity(nc, ident)

            vp4 = vplus.rearrange("p (t g c) -> p t g c", t=NT, g=HL, c=VW)
            nc.gpsimd.memset(vp4[:, :, :, DH], 1.0)

            # Prime the DVE vector clock on the mask DMA so the first
            # tensor_mul needs only the PE wait.
            scratch = cp.tile([1, 1], f32, tag="scratch")
            nc.vector.tensor_copy(scratch, qkm_sb[0:1, 0:1])

            with (
                tc.tile_pool(name="psqk", bufs=2, space="PSUM") as pqk,
                tc.tile_pool(name="pss", bufs=2, space="PSUM") as pss,
                tc.tile_pool(name="psa", bufs=1, space="PSUM") as psa,
                tc.tile_pool(name="tpool", bufs=3) as tp,
                tc.tile_pool(name="spool", bufs=2) as sp,
            ):

                def emit_qk(pair):
                    # q f-tile `pair` and k f-tile NP+pair, in N-quarters.
                    for ft in (pair, NP + pair):
                        for qu in range(4):
                            ps_qk = pqk.tile([P, 512], f32, tag="qk", name="ps_qk")
                            for dmt in range(DMT):
                                nc.tensor.matmul(
                                    ps_qk[:, :],
                                    wqk_sb[
                                        :, dmt * FQK + ft * P : dmt * FQK + (ft + 1) * P
                                    ],
                                    xT_sb[
                                        :, dmt * N + qu * 512 : dmt * N + (qu + 1) * 512
                                    ],
                                    start=(dmt == 0),
                                    stop=(dmt == DMT - 1),
                                )
                            nc.vector.tensor_mul(
                                qk_all[:, ft * N + qu * 512 : ft * N + (qu + 1) * 512],
                                ps_qk[:, :],
                                qkm_sb[:, ts(qu, 512)],
                            )

                emit_qk(0)

                # ---- V projection (token-major) + masked-v row C ----
                for tt in range(NT):
                    ps_v = pqk.tile([P, FV], f32, tag="qk", name="ps_v")
                    for dmt in range(DMT):
                        nc.tensor.matmul(
                            ps_v[:, :],
                            xT_sb[:, dmt * N + tt * P : dmt * N + (tt + 1) * P],
                            wv_sb[:, ts(dmt, FV)],
                            start=(dmt == 0),
                            stop=(dmt == DMT - 1),
                        )
                    nc.vector.tensor_copy(
                        vp4[:, tt, :, 0:DH],
                        ps_v.rearrange("p (g c) -> p g c", g=HL, c=DH),
                    )
                ps_c0 = pss.tile([1, VROW // 2], f32, tag="s", name="ps_c0")
                ps_c1 = pss.tile([1, VROW // 2], f32, tag="s", name="ps_c1")
                for jt in range(NT):
                    nc.tensor.matmul(
                        ps_c0[:, :],
                        iminv_sb[:, jt : jt + 1],
                        vplus[:, jt * VROW : jt * VROW + VROW // 2],
                        start=(jt == 0),
                        stop=(jt == NT - 1),
                    )
                    nc.tensor.matmul(
                        ps_c1[:, :],
                        iminv_sb[:, jt : jt + 1],
                        vplus[:, jt * VROW + VROW // 2 : (jt + 1) * VROW],
                        start=(jt == 0),
                        stop=(jt == NT - 1),
                    )
                nc.vector.tensor_scalar_mul(c_sb[:, 0 : VROW // 2], ps_c0[:, :], -1.0)
                nc.vector.tensor_scalar_mul(
                    c_sb[:, VROW // 2 : VROW], ps_c1[:, :], -1.0
                )

                # ---- head loop: scores^T -> exp -> PV -> normalize ----
                for pair in range(NP):
                    for hh in range(2):
                        h = 2 * pair + hh
                        p0 = hh * 64
                        qcol = pair * N
                        kcol = (NP + pair) * N
                        for ih in range(2):
                            pa = psa.tile([P, NH], f32, tag="att", name="pa")
                            for jt in range(NT):
                                t_sb = tp.tile([P, NH], bf, tag="t", name="t_sb")
                                kT = qk_all[
                                    p0 : p0 + 64, kcol + jt * P : kcol + (jt + 1) * P
                                ]
                                ps_s = pss.tile([P, NH], f32, tag="s", name="ps_s")
                                for ch in range(2):
                                    c0 = qcol + ih * NH + ch * 512
                                    nc.tensor.matmul(
                                        ps_s[:, ts(ch, 512)],
                                        kT,
                                        qk_all[p0 : p0 + 64, c0 : c0 + 512],
                                        start=True,
                                        stop=True,
                                    )
                                nc.scalar.activation(t_sb[:, :], ps_s[:, :], EXP)
                                vslice = vplus[
                                    :, jt * VROW + h * VW : jt * VROW + (h + 1) * VW
                                ]
                                for it8 in range(NHT):
                                    nc.tensor.matmul(
                                        pa[:, it8 * P : it8 * P + VW],
                                        t_sb[:, ts(it8, P)],
                                        vslice,
                                        start=(jt == 0 and it8 % 4 == 0),
                                        stop=False,
                                    )
                            for it8 in range(NHT):
                                nc.tensor.matmul(
                                    pa[:, it8 * P : it8 * P + VW],
                                    mrow_sb[
                                        :, ih * NH + it8 * P : ih * NH + (it8 + 1) * P
                                    ],
                                    c_sb[:, h * VW : (h + 1) * VW],
                                    start=False,
                                    stop=(it8 % 4 == 3),
                                )
                            r_sb = sp.tile([P, NHT], f32, tag="r", name="r_sb")
                            pa3 = pa.rearrange("p (t c) -> p t c", t=NHT, c=P)
                            nc.vector.reciprocal(r_sb[:, :], pa3[:, :, DH])
                            for it8 in range(NHT):
                                it = ih * NHT + it8
                                dst = att_pair[
                                    :, it * P + p0 : it * P + p0 + DH
                                ]
                                nc.vector.tensor_scalar_mul(
                                    dst,
                                    pa[:, it8 * P : it8 * P + DH],
                                    r_sb[:, it8 : it8 + 1],
                                )
                    for it in range(NT):
                        ps_tr = pss.tile([P, P], bf, tag="s", name="ps_tr")
                        nc.tensor.transpose(ps_tr[:, :], att_pair[:, ts(it, P)], ident)
                        nc.vector.tensor_copy(
                            attT[:, pair * N + it * P : pair * N + (it + 1) * P],
                            ps_tr[:, :],
                        )
                    if pair + 1 < NP:
                        emit_qk(pair + 1)

                # ---- partial output projection ----
                # [128, 512] chunks so the tiles fit the 1-bank "qk" slots.
                # PSUM->SBUF eviction on the Scalar engine (idle here).
                COPY = mybir.ActivationFunctionType.Copy
                for it in range(NT):
                    for ch in range(2):
                        ps_o = pqk.tile([P, 512], f32, tag="qk", name="ps_o")
                        for ht in range(HT):
                            nc.tensor.matmul(
                                ps_o[:, :],
                                attT[:, ht * N + it * P : ht * N + (it + 1) * P],
                                wout_sb[
                                    :, ht * DM + ch * 512 : ht * DM + (ch + 1) * 512
                                ],
                                start=(ht == 0),
                                stop=(ht == HT - 1),
                            )
                        o_sb = sp.tile([P, 512], f32, tag="ob", name="o_sb")
                        nc.scalar.activation(o_sb[:, :], ps_o[:, :], COPY)
                        nc.sync.dma_start(
                            out=out[ts(it, P), ts(ch, 512)], in_=o_sb[:, :]
                        )

    nc.compile()
    return nc


def _shard_inputs(x, w_qkv, w_out, b_out, mask):
    """Build the per-core input maps (host-side sharding + layout prep)."""
    bf = ml_dtypes.bfloat16
    x = np.asarray(x, dtype=np.float32)
    w_qkv = np.asarray(w_qkv, dtype=np.float32)
    w_out = np.asarray(w_out, dtype=np.float32)
    mask = np.asarray(mask)

    # w_qkv columns: head h occupies cols [h*192, (h+1)*192) as q|k|v of 64.
    w3 = w_qkv.reshape(DM, H, 3, DH)
    in_maps = []
    for c in range(NCORES):
        b, hg = c // HG, c % HG
        # q features for all 8 heads (cols 0:512, pre-scaled by SCALE),
        # then k features
        wqk_c = np.ascontiguousarray(
            np.concatenate(
                [
                    w3[:, hg * HL : (hg + 1) * HL, 0, :].reshape(DM, FV) * SCALE,
                    w3[:, hg * HL : (hg + 1) * HL, 1, :].reshape(DM, FV),
                ],
                axis=1,
            )
        ).astype(bf)
        wv_c = np.ascontiguousarray(
            w3[:, hg * HL : (hg + 1) * HL, 2, :].reshape(DM, FV)
        ).astype(bf)
        wout_c = np.ascontiguousarray(w_out[hg * FV : (hg + 1) * FV, :]).astype(bf)
        xT_c = np.ascontiguousarray(x[b].T).astype(bf)

        m = mask[b].astype(np.float32)  # [N] of 0/1
        qkm = np.broadcast_to(m[None, :], (P, N)).copy()
        mrow_c = m[None, :].astype(bf)
        iminv_c = np.ascontiguousarray((1.0 - m).reshape(NT, P).T).astype(bf)

        in_maps.append(
            {
                "xT": xT_c,
                "wqk": wqk_c,
                "wv": wv_c,
                "wout": wout_c,
                "qkmask": qkm,
                "mrow": mrow_c,
                "iminv": iminv_c,
            }
        )
    return in_maps


def kernel(x, w_qkv, w_out, b_out, mask):
    from concourse.bass_utils import run_bass_kernel_spmd

    if "nc" not in _CACHE:
        _CACHE["nc"] = _build_program()
    nc = _CACHE["nc"]

    in_maps = _shard_inputs(x, w_qkv, w_out, b_out, mask)
    res = run_bass_kernel_spmd(nc, in_maps, list(range(NCORES))).results

    b_out = np.asarray(b_out, dtype=np.float32)
    out = np.empty((B, N, DM), np.float32)
    for b in range(B):
        out[b] = res[HG * b]["out"] + res[HG * b + 1]["out"] + b_out[None, :]
    return out


# revision 27
# speedup vs baseline: 1.3107x; 1.1734x over previous
"""Multi-head attention (B=4, N=2048, DM=1024, H=16, DH=64) on 8 trn2 cores.

Sharding: core c -> (batch b = c//2, head-group hg = c%2 of 8 heads).
Each core computes qkv for its 8 heads, masked softmax attention, and a
partial output projection over its 512 head-dims.  Host sums the two
partials per batch and adds the bias.

Device-side layout ("feature-major"):
  - x^T [DM, N] so QK projection emits q^T/k^T [64, N] per head directly.
  - mask folded into q^T (x SCALE*m_i, SCALE pre-baked in w_q) and k^T
    (x m_j): masked score pairs become exp(0)=1; a rank-1 correction
    matmul (-m_i * C_h, with C_h = sum_{masked j} v_h[j,:]) cancels them
    for live queries, and dead queries (m_i=0) fall out as the exact
    uniform-softmax rows the reference produces.
  - v stored token-major with an appended ones column per head, so the
    PV matmul accumulates the softmax denominator for free.
  - per-head-pair QK projection is interleaved into the head loop so the
    PE keeps busy while ACT (exp) is the softmax bottleneck.
"""

import sys

sys.path.insert(0, "/opt/trn_rl_repo")

import numpy as np
import ml_dtypes

B, N, DM, H, DH = 4, 2048, 1024, 16, 64
SCALE = DH**-0.5
NCORES = 8
HG = 2  # head groups (tensor-parallel factor)
HL = H // HG  # 8 heads per core
NP = HL // 2  # 4 head pairs
FQK = HL * 2 * DH  # 1024 qk features per core
FV = HL * DH  # 512 v features per core
P = 128
NT = N // P  # 16 token tiles
DMT = DM // P  # 8 dm tiles
VW = DH + 1  # 65: v columns + ones column
VROW = HL * VW  # 520
HT = FV // P  # 4 head-dim tiles for the projection
NH = N // 2  # 1024: i-half width
NHT = NT // 2  # 8 token tiles per i-half

_CACHE = {}


def _build_program():
    import concourse.mybir as mybir
    import concourse.tile as tile
    from concourse import bacc
    from concourse.bass import ts
    from concourse.masks import make_identity

    bf = mybir.dt.bfloat16
    f32 = mybir.dt.float32
    EXP = mybir.ActivationFunctionType.Exp

    nc = bacc.Bacc(
        "TRN2", target_bir_lowering=False, debug=False, num_devices=NCORES
    )
    xT = nc.dram_tensor("xT", [DM, N], bf, kind="ExternalInput")
    wqk = nc.dram_tensor("wqk", [DM, FQK], bf, kind="ExternalInput")
    wv = nc.dram_tensor("wv", [DM, FV], bf, kind="ExternalInput")
    wout = nc.dram_tensor("wout", [FV, DM], bf, kind="ExternalInput")
    qkmask = nc.dram_tensor("qkmask", [P, N], f32, kind="ExternalInput")
    mrow = nc.dram_tensor("mrow", [1, N], bf, kind="ExternalInput")
    iminv = nc.dram_tensor("iminv", [P, NT], bf, kind="ExternalInput")
    out = nc.dram_tensor("out", [N, DM], f32, kind="ExternalOutput")

    with tile.TileContext(nc) as tc:
        with tc.tile_pool(name="const", bufs=1) as cp:
            xT_sb = cp.tile([P, DMT * N], bf, tag="xT")
            wqk_sb = cp.tile([P, DMT * FQK], bf, tag="wqk")
            wv_sb = cp.tile([P, DMT * FV], bf, tag="wv")
            wout_sb = cp.tile([P, HT * DM], bf, tag="wout")
            qkm_sb = cp.tile([P, N], f32, tag="qkm")
            mrow_sb = cp.tile([1, N], bf, tag="mrow")
            iminv_sb = cp.tile([P, NT], bf, tag="iminv")
            ident = cp.tile([P, P], bf, tag="ident")
            vplus = cp.tile([P, NT * VROW], bf, tag="vplus")
            qk_all = cp.tile([P, HL * N], bf, tag="qkall")
            attT = cp.tile([P, HT * N], bf, tag="attT")
            att_pair = cp.tile([P, NT * P], bf, tag="attpair")
            c_sb = cp.tile([1, VROW], bf, tag="csb")

            # Small mask tensors first: the DVE program's first op waits on
            # qkmask, so it must not queue behind the big weight DMAs.
            nc.sync.dma_start(out=qkm_sb[:, :], in_=qkmask[:, :])
            nc.sync.dma_start(out=mrow_sb[:, :], in_=mrow[:, :])
            nc.sync.dma_start(out=iminv_sb[:, :], in_=iminv[:, :])
            for dmt in range(DMT):
                nc.sync.dma_start(out=xT_sb[:, ts(dmt, N)], in_=xT[ts(dmt, P), :])
                nc.sync.dma_start(out=wqk_sb[:, ts(dmt, FQK)], in_=wqk[ts(dmt, P), :])
                nc.sync.dma_start(out=wv_sb[:, ts(dmt, FV)], in_=wv[ts(dmt, P), :])
            for ht in range(HT):
                nc.sync.dma_start(out=wout_sb[:, ts(ht, DM)], in_=wout[ts(ht, P), :])
            make_identity(nc, ident)

            vp4 = vplus.rearrange("p (t g c) -> p t g c", t=NT, g=HL, c=VW)
            nc.gpsimd.memset(vp4[:, :, :, DH], 1.0)

            # Prime the DVE vector clock on the mask DMA so the first
            # tensor_mul needs only the PE wait.
            scratch = cp.tile([1, 1], f32, tag="scratch")
            nc.vector.tensor_copy(scratch, qkm_sb[0:1, 0:1])

            with (
                tc.tile_pool(name="psqk", bufs=2, space="PSUM") as pqk,
                tc.tile_pool(name="pss", bufs=2, space="PSUM") as pss,
                tc.tile_pool(name="psa", bufs=1, space="PSUM") as psa,
                tc.tile_pool(name="tpool", bufs=17) as tp,
                tc.tile_pool(name="spool", bufs=4) as sp,
            ):

                def emit_qk_chunk(ft, qu):
                    ps_qk = pqk.tile([P, 512], f32, tag="qk", name="ps_qk")
                    for dmt in range(DMT):
                        nc.tensor.matmul(
                            ps_qk[:, :],
                            wqk_sb[:, dmt * FQK + ft * P : dmt * FQK + (ft + 1) * P],
                            xT_sb[:, dmt * N + qu * 512 : dmt * N + (qu + 1) * 512],
                            start=(dmt == 0),
                            stop=(dmt == DMT - 1),
                        )
                    nc.vector.tensor_mul(
                        qk_all[:, ft * N + qu * 512 : ft * N + (qu + 1) * 512],
                        ps_qk[:, :],
                        qkm_sb[:, ts(qu, 512)],
                    )

                def emit_qk(pair):
                    # q f-tile `pair` and k f-tile NP+pair, in N-quarters.
                    for ft in (pair, NP + pair):
                        for qu in range(4):
                            emit_qk_chunk(ft, qu)

                emit_qk(0)

                # Pre-emit head 0 / i-half 0 scores+exp ahead of the V
                # projection so ACT starts working ~16us in instead of
                # waiting for V+C (~60us).  The t tiles are consumed by the
                # regular PV loop below once vplus is ready.
                pre_t = []
                for jt in range(NT):
                    t_sb = tp.tile([P, NH], bf, tag="t", name="t_sb")
                    kT = qk_all[0:64, NP * N + jt * P : NP * N + (jt + 1) * P]
                    ps_s = pss.tile([P, NH], f32, tag="s", name="ps_s")
                    for ch in range(2):
                        nc.tensor.matmul(
                            ps_s[:, ts(ch, 512)],
                            kT,
                            qk_all[0:64, ch * 512 : (ch + 1) * 512],
                            start=True,
                            stop=True,
                        )
                    nc.scalar.activation(t_sb[:, :], ps_s[:, :], EXP)
                    pre_t.append(t_sb)

                # ---- V projection (token-major) + masked-v row C ----
                for tt in range(NT):
                    ps_v = pqk.tile([P, FV], f32, tag="qk", name="ps_v")
                    for dmt in range(DMT):
                        nc.tensor.matmul(
                            ps_v[:, :],
                            xT_sb[:, dmt * N + tt * P : dmt * N + (tt + 1) * P],
                            wv_sb[:, ts(dmt, FV)],
                            start=(dmt == 0),
                            stop=(dmt == DMT - 1),
                        )
                    nc.vector.tensor_copy(
                        vp4[:, tt, :, 0:DH],
                        ps_v.rearrange("p (g c) -> p g c", g=HL, c=DH),
                    )
                # C tiles live in the 1-bank "qk" slots: putting them in the
                # scores pool would pin both scores slots until the whole V
                # projection finishes, stalling ACT ~76us at startup.
                ps_c0 = pqk.tile([1, VROW // 2], f32, tag="qk", name="ps_c0")
                ps_c1 = pqk.tile([1, VROW // 2], f32, tag="qk", name="ps_c1")
                for jt in range(NT):
                    nc.tensor.matmul(
                        ps_c0[:, :],
                        iminv_sb[:, jt : jt + 1],
                        vplus[:, jt * VROW : jt * VROW + VROW // 2],
                        start=(jt == 0),
                        stop=(jt == NT - 1),
                    )
                    nc.tensor.matmul(
                        ps_c1[:, :],
                        iminv_sb[:, jt : jt + 1],
                        vplus[:, jt * VROW + VROW // 2 : (jt + 1) * VROW],
                        start=(jt == 0),
                        stop=(jt == NT - 1),
                    )
                nc.vector.tensor_scalar_mul(c_sb[:, 0 : VROW // 2], ps_c0[:, :], -1.0)
                nc.vector.tensor_scalar_mul(
                    c_sb[:, VROW // 2 : VROW], ps_c1[:, :], -1.0
                )

                # ---- head loop: scores^T -> exp -> PV -> normalize ----
                # Next pair's QK projection is spread 2 chunks per
                # (head, i-half) block so the PE fills its slack inside the
                # ACT-bound softmax phase instead of stalling ACT at pair
                # boundaries.
                for pair in range(NP):
                    next_chunks = (
                        [(ft, qu) for ft in (pair + 1, NP + pair + 1) for qu in range(4)]
                        if pair + 1 < NP
                        else []
                    )
                    blk = 0
                    for hh in range(2):
                        h = 2 * pair + hh
                        p0 = hh * 64
                        qcol = pair * N
                        kcol = (NP + pair) * N
                        for ih in range(2):
                            pa = psa.tile([P, NH], f32, tag="att", name="pa")
                            for jt in range(NT):
                                if h == 0 and ih == 0:
                                    t_sb = pre_t[jt]
                                else:
                                    t_sb = tp.tile([P, NH], bf, tag="t", name="t_sb")
                                    kT = qk_all[
                                        p0 : p0 + 64,
                                        kcol + jt * P : kcol + (jt + 1) * P,
                                    ]
                                    ps_s = pss.tile([P, NH], f32, tag="s", name="ps_s")
                                    for ch in range(2):
                                        c0 = qcol + ih * NH + ch * 512
                                        nc.tensor.matmul(
                                            ps_s[:, ts(ch, 512)],
                                            kT,
                                            qk_all[p0 : p0 + 64, c0 : c0 + 512],
                                            start=True,
                                            stop=True,
                                        )
                                    nc.scalar.activation(t_sb[:, :], ps_s[:, :], EXP)
                                vslice = vplus[
                                    :, jt * VROW + h * VW : jt * VROW + (h + 1) * VW
                                ]
                                for it8 in range(NHT):
                                    nc.tensor.matmul(
                                        pa[:, it8 * P : it8 * P + VW],
                                        t_sb[:, ts(it8, P)],
                                        vslice,
                                        start=(jt == 0 and it8 % 4 == 0),
                                        stop=False,
                                    )
                            for it8 in range(NHT):
                                nc.tensor.matmul(
                                    pa[:, it8 * P : it8 * P + VW],
                                    mrow_sb[
                                        :, ih * NH + it8 * P : ih * NH + (it8 + 1) * P
                                    ],
                                    c_sb[:, h * VW : (h + 1) * VW],
                                    start=False,
                                    stop=(it8 % 4 == 3),
                                )
                            r_sb = sp.tile([P, NHT], f32, tag="r", name="r_sb")
                            pa3 = pa.rearrange("p (t c) -> p t c", t=NHT, c=P)
                            nc.vector.reciprocal(r_sb[:, :], pa3[:, :, DH])
                            for it8 in range(NHT):
                                it = ih * NHT + it8
                                dst = att_pair[
                                    :, it * P + p0 : it * P + p0 + DH
                                ]
                                nc.vector.tensor_scalar_mul(
                                    dst,
                                    pa[:, it8 * P : it8 * P + DH],
                                    r_sb[:, it8 : it8 + 1],
                                )
                            for ft_qu in next_chunks[2 * blk : 2 * blk + 2]:
                                emit_qk_chunk(*ft_qu)
                            blk += 1
                    for it in range(NT):
                        ps_tr = pqk.tile([P, P], bf, tag="qk", name="ps_tr")
                        nc.tensor.transpose(ps_tr[:, :], att_pair[:, ts(it, P)], ident)
                        nc.vector.tensor_copy(
                            attT[:, pair * N + it * P : pair * N + (it + 1) * P],
                            ps_tr[:, :],
                        )

                # ---- partial output projection ----
                # [128, 512] chunks so the tiles fit the 1-bank "qk" slots.
                # PSUM->SBUF eviction on the Scalar engine (idle here).
                COPY = mybir.ActivationFunctionType.Copy
                for it in range(NT):
                    for ch in range(2):
                        ps_o = pqk.tile([P, 512], f32, tag="qk", name="ps_o")
                        for ht in range(HT):
                            nc.tensor.matmul(
                                ps_o[:, :],
                                attT[:, ht * N + it * P : ht * N + (it + 1) * P],
                                wout_sb[
                                    :, ht * DM + ch * 512 : ht * DM + (ch + 1) * 512
                                ],
                                start=(ht == 0),
                                stop=(ht == HT - 1),
                            )
                        o_sb = sp.tile([P, 512], f32, tag="ob", name="o_sb")
                        # Alternate eviction engine so ACT and DVE each
                        # drain half the projection chunks in parallel.
                        if ch == 0:
                            nc.scalar.activation(o_sb[:, :], ps_o[:, :], COPY)
                        else:
                            nc.vector.tensor_copy(o_sb[:, :], ps_o[:, :])
                        nc.sync.dma_start(
                            out=out[ts(it, P), ts(ch, 512)], in_=o_sb[:, :]
                        )

    nc.compile()
    return nc


def _shard_inputs(x, w_qkv, w_out, b_out, mask):
    """Build the per-core input maps (host-side sharding + layout prep)."""
    bf = ml_dtypes.bfloat16
    x = np.asarray(x, dtype=np.float32)
    w_qkv = np.asarray(w_qkv, dtype=np.float32)
    w_out = np.asarray(w_out, dtype=np.float32)
    mask = np.asarray(mask)

    # w_qkv columns: head h occupies cols [h*192, (h+1)*192) as q|k|v of 64.
    w3 = w_qkv.reshape(DM, H, 3, DH)
    in_maps = []
    for c in range(NCORES):
        b, hg = c // HG, c % HG
        # q features for all 8 heads (cols 0:512, pre-scaled by SCALE),
        # then k features
        wqk_c = np.ascontiguousarray(
            np.concatenate(
                [
                    w3[:, hg * HL : (hg + 1) * HL, 0, :].reshape(DM, FV) * SCALE,
                    w3[:, hg * HL : (hg + 1) * HL, 1, :].reshape(DM, FV),
                ],
                axis=1,
            )
        ).astype(bf)
        wv_c = np.ascontiguousarray(
            w3[:, hg * HL : (hg + 1) * HL, 2, :].reshape(DM, FV)
        ).astype(bf)
        wout_c = np.ascontiguousarray(w_out[hg * FV : (hg + 1) * FV, :]).astype(bf)
        xT_c = np.ascontiguousarray(x[b].T).astype(bf)

        m = mask[b].astype(np.float32)  # [N] of 0/1
        qkm = np.broadcast_to(m[None, :], (P, N)).copy()
        mrow_c = m[None, :].astype(bf)
        iminv_c = np.ascontiguousarray((1.0 - m).reshape(NT, P).T).astype(bf)

        in_maps.append(
            {
                "xT": xT_c,
                "wqk": wqk_c,
                "wv": wv_c,
                "wout": wout_c,
                "qkmask": qkm,
                "mrow": mrow_c,
                "iminv": iminv_c,
            }
        )
    return in_maps


def kernel(x, w_qkv, w_out, b_out, mask):
    from concourse.bass_utils import run_bass_kernel_spmd

    if "nc" not in _CACHE:
        _CACHE["nc"] = _build_program()
    nc = _CACHE["nc"]

    in_maps = _shard_inputs(x, w_qkv, w_out, b_out, mask)
    res = run_bass_kernel_spmd(nc, in_maps, list(range(NCORES))).results

    b_out = np.asarray(b_out, dtype=np.float32)
    out = np.empty((B, N, DM), np.float32)
    for b in range(B):
        out[b] = res[HG * b]["out"] + res[HG * b + 1]["out"] + b_out[None, :]
    return out


# revision 28
# speedup vs baseline: 1.3196x; 1.0067x over previous
"""Multi-head attention (B=4, N=2048, DM=1024, H=16, DH=64) on 8 trn2 cores.

Sharding: core c -> (batch b = c//2, head-group hg = c%2 of 8 heads).
Each core computes qkv for its 8 heads, masked softmax attention, and a
partial output projection over its 512 head-dims.  Host sums the two
partials per batch and adds the bias.

Device-side layout ("feature-major"):
  - x^T [DM, N] so QK projection emits q^T/k^T [64, N] per head directly.
  - mask folded into q^T (x SCALE*m_i, SCALE pre-baked in w_q) and k^T
    (x m_j): masked score pairs become exp(0)=1; a rank-1 correction
    matmul (-m_i * C_h, with C_h = sum_{masked j} v_h[j,:]) cancels them
    for live queries, and dead queries (m_i=0) fall out as the exact
    uniform-softmax rows the reference produces.
  - v stored token-major with an appended ones column per head, so the
    PV matmul accumulates the softmax denominator for free.
  - per-head-pair QK projection is interleaved into the head loop so the
    PE keeps busy while ACT (exp) is the softmax bottleneck.
"""

import sys

sys.path.insert(0, "/opt/trn_rl_repo")

import numpy as np
import ml_dtypes

B, N, DM, H, DH = 4, 2048, 1024, 16, 64
SCALE = DH**-0.5
NCORES = 8
HG = 2  # head groups (tensor-parallel factor)
HL = H // HG  # 8 heads per core
NP = HL // 2  # 4 head pairs
FQK = HL * 2 * DH  # 1024 qk features per core
FV = HL * DH  # 512 v features per core
P = 128
NT = N // P  # 16 token tiles
DMT = DM // P  # 8 dm tiles
VW = DH + 1  # 65: v columns + ones column
VROW = HL * VW  # 520
HT = FV // P  # 4 head-dim tiles for the projection
NH = N // 2  # 1024: i-half width
NHT = NT // 2  # 8 token tiles per i-half

_CACHE = {}


def _build_program():
    import concourse.mybir as mybir
    import concourse.tile as tile
    from concourse import bacc
    from concourse.bass import ts
    from concourse.masks import make_identity

    bf = mybir.dt.bfloat16
    f32 = mybir.dt.float32
    EXP = mybir.ActivationFunctionType.Exp

    nc = bacc.Bacc(
        "TRN2", target_bir_lowering=False, debug=False, num_devices=NCORES
    )
    xT = nc.dram_tensor("xT", [DM, N], bf, kind="ExternalInput")
    wqk = nc.dram_tensor("wqk", [DM, FQK], bf, kind="ExternalInput")
    wv = nc.dram_tensor("wv", [DM, FV], bf, kind="ExternalInput")
    wout = nc.dram_tensor("wout", [FV, DM], bf, kind="ExternalInput")
    qkmask = nc.dram_tensor("qkmask", [P, N], f32, kind="ExternalInput")
    mrow = nc.dram_tensor("mrow", [1, N], bf, kind="ExternalInput")
    iminv = nc.dram_tensor("iminv", [P, NT], bf, kind="ExternalInput")
    out = nc.dram_tensor("out", [N, DM], f32, kind="ExternalOutput")

    with tile.TileContext(nc) as tc:
        with tc.tile_pool(name="const", bufs=1) as cp:
            xT_sb = cp.tile([P, DMT * N], bf, tag="xT")
            wqk_sb = cp.tile([P, DMT * FQK], bf, tag="wqk")
            wv_sb = cp.tile([P, DMT * FV], bf, tag="wv")
            wout_sb = cp.tile([P, HT * DM], bf, tag="wout")
            qkm_sb = cp.tile([P, N], f32, tag="qkm")
            mrow_sb = cp.tile([1, N], bf, tag="mrow")
            iminv_sb = cp.tile([P, NT], bf, tag="iminv")
            ident = cp.tile([P, P], bf, tag="ident")
            vplus = cp.tile([P, NT * VROW], bf, tag="vplus")
            qk_all = cp.tile([P, HL * N], bf, tag="qkall")
            attT = cp.tile([P, HT * N], bf, tag="attT")
            att_pair = cp.tile([P, NT * P], bf, tag="attpair")
            c_sb = cp.tile([1, VROW], bf, tag="csb")

            # DMA order mirrors consumption: the first QK chunk's inputs
            # (xT/wqk dmt 0) lead, then the small mask tensors (the DVE
            # program's first op waits on qkmask), then the remaining
            # xT/wqk tiles; wv and wout are needed later so they go last.
            nc.sync.dma_start(out=xT_sb[:, ts(0, N)], in_=xT[ts(0, P), :])
            nc.sync.dma_start(out=wqk_sb[:, ts(0, FQK)], in_=wqk[ts(0, P), :])
            nc.sync.dma_start(out=qkm_sb[:, :], in_=qkmask[:, :])
            nc.sync.dma_start(out=mrow_sb[:, :], in_=mrow[:, :])
            nc.sync.dma_start(out=iminv_sb[:, :], in_=iminv[:, :])
            for dmt in range(1, DMT):
                nc.sync.dma_start(out=xT_sb[:, ts(dmt, N)], in_=xT[ts(dmt, P), :])
                nc.sync.dma_start(out=wqk_sb[:, ts(dmt, FQK)], in_=wqk[ts(dmt, P), :])
            for dmt in range(DMT):
                nc.sync.dma_start(out=wv_sb[:, ts(dmt, FV)], in_=wv[ts(dmt, P), :])
            for ht in range(HT):
                nc.sync.dma_start(out=wout_sb[:, ts(ht, DM)], in_=wout[ts(ht, P), :])
            make_identity(nc, ident)

            vp4 = vplus.rearrange("p (t g c) -> p t g c", t=NT, g=HL, c=VW)
            nc.gpsimd.memset(vp4[:, :, :, DH], 1.0)

            # Prime the DVE vector clock on the mask DMA so the first
            # tensor_mul needs only the PE wait.
            scratch = cp.tile([1, 1], f32, tag="scratch")
            nc.vector.tensor_copy(scratch, qkm_sb[0:1, 0:1])

            with (
                tc.tile_pool(name="psqk", bufs=2, space="PSUM") as pqk,
                tc.tile_pool(name="pss", bufs=2, space="PSUM") as pss,
                tc.tile_pool(name="psa", bufs=1, space="PSUM") as psa,
                tc.tile_pool(name="tpool", bufs=17) as tp,
                tc.tile_pool(name="spool", bufs=4) as sp,
            ):

                def emit_qk_chunk(ft, qu):
                    ps_qk = pqk.tile([P, 512], f32, tag="qk", name="ps_qk")
                    for dmt in range(DMT):
                        nc.tensor.matmul(
                            ps_qk[:, :],
                            wqk_sb[:, dmt * FQK + ft * P : dmt * FQK + (ft + 1) * P],
                            xT_sb[:, dmt * N + qu * 512 : dmt * N + (qu + 1) * 512],
                            start=(dmt == 0),
                            stop=(dmt == DMT - 1),
                        )
                    nc.vector.tensor_mul(
                        qk_all[:, ft * N + qu * 512 : ft * N + (qu + 1) * 512],
                        ps_qk[:, :],
                        qkm_sb[:, ts(qu, 512)],
                    )

                def emit_qk(pair):
                    # q f-tile `pair` and k f-tile NP+pair, in N-quarters.
                    for ft in (pair, NP + pair):
                        for qu in range(4):
                            emit_qk_chunk(ft, qu)

                emit_qk(0)

                # Pre-emit head 0 / i-half 0 scores+exp ahead of the V
                # projection so ACT starts working ~16us in instead of
                # waiting for V+C (~60us).  The t tiles are consumed by the
                # regular PV loop below once vplus is ready.
                pre_t = []
                for jt in range(NT):
                    t_sb = tp.tile([P, NH], bf, tag="t", name="t_sb")
                    kT = qk_all[0:64, NP * N + jt * P : NP * N + (jt + 1) * P]
                    ps_s = pss.tile([P, NH], f32, tag="s", name="ps_s")
                    for ch in range(2):
                        nc.tensor.matmul(
                            ps_s[:, ts(ch, 512)],
                            kT,
                            qk_all[0:64, ch * 512 : (ch + 1) * 512],
                            start=True,
                            stop=True,
                        )
                    nc.scalar.activation(t_sb[:, :], ps_s[:, :], EXP)
                    pre_t.append(t_sb)

                # ---- V projection (token-major) + masked-v row C ----
                for tt in range(NT):
                    ps_v = pqk.tile([P, FV], f32, tag="qk", name="ps_v")
                    for dmt in range(DMT):
                        nc.tensor.matmul(
                            ps_v[:, :],
                            xT_sb[:, dmt * N + tt * P : dmt * N + (tt + 1) * P],
                            wv_sb[:, ts(dmt, FV)],
                            start=(dmt == 0),
                            stop=(dmt == DMT - 1),
                        )
                    nc.vector.tensor_copy(
                        vp4[:, tt, :, 0:DH],
                        ps_v.rearrange("p (g c) -> p g c", g=HL, c=DH),
                    )
                # C tiles live in the 1-bank "qk" slots: putting them in the
                # scores pool would pin both scores slots until the whole V
                # projection finishes, stalling ACT ~76us at startup.
                ps_c0 = pqk.tile([1, VROW // 2], f32, tag="qk", name="ps_c0")
                ps_c1 = pqk.tile([1, VROW // 2], f32, tag="qk", name="ps_c1")
                for jt in range(NT):
                    nc.tensor.matmul(
                        ps_c0[:, :],
                        iminv_sb[:, jt : jt + 1],
                        vplus[:, jt * VROW : jt * VROW + VROW // 2],
                        start=(jt == 0),
                        stop=(jt == NT - 1),
                    )
                    nc.tensor.matmul(
                        ps_c1[:, :],
                        iminv_sb[:, jt : jt + 1],
                        vplus[:, jt * VROW + VROW // 2 : (jt + 1) * VROW],
                        start=(jt == 0),
                        stop=(jt == NT - 1),
                    )
                nc.vector.tensor_scalar_mul(c_sb[:, 0 : VROW // 2], ps_c0[:, :], -1.0)
                nc.vector.tensor_scalar_mul(
                    c_sb[:, VROW // 2 : VROW], ps_c1[:, :], -1.0
                )

                # ---- head loop: scores^T -> exp -> PV -> normalize ----
                # Next pair's QK projection is spread 2 chunks per
                # (head, i-half) block so the PE fills its slack inside the
                # ACT-bound softmax phase instead of stalling ACT at pair
                # boundaries.
                for pair in range(NP):
                    next_chunks = (
                        [(ft, qu) for ft in (pair + 1, NP + pair + 1) for qu in range(4)]
                        if pair + 1 < NP
                        else []
                    )
                    blk = 0
                    for hh in range(2):
                        h = 2 * pair + hh
                        p0 = hh * 64
                        qcol = pair * N
                        kcol = (NP + pair) * N
                        for ih in range(2):
                            pa = psa.tile([P, NH], f32, tag="att", name="pa")
                            for jt in range(NT):
                                if h == 0 and ih == 0:
                                    t_sb = pre_t[jt]
                                else:
                                    t_sb = tp.tile([P, NH], bf, tag="t", name="t_sb")
                                    kT = qk_all[
                                        p0 : p0 + 64,
                                        kcol + jt * P : kcol + (jt + 1) * P,
                                    ]
                                    ps_s = pss.tile([P, NH], f32, tag="s", name="ps_s")
                                    for ch in range(2):
                                        c0 = qcol + ih * NH + ch * 512
                                        nc.tensor.matmul(
                                            ps_s[:, ts(ch, 512)],
                                            kT,
                                            qk_all[p0 : p0 + 64, c0 : c0 + 512],
                                            start=True,
                                            stop=True,
                                        )
                                    nc.scalar.activation(t_sb[:, :], ps_s[:, :], EXP)
                                vslice = vplus[
                                    :, jt * VROW + h * VW : jt * VROW + (h + 1) * VW
                                ]
                                for it8 in range(NHT):
                                    nc.tensor.matmul(
                                        pa[:, it8 * P : it8 * P + VW],
                                        t_sb[:, ts(it8, P)],
                                        vslice,
                                        start=(jt == 0 and it8 % 4 == 0),
                                        stop=False,
                                    )
                            for it8 in range(NHT):
                                nc.tensor.matmul(
                                    pa[:, it8 * P : it8 * P + VW],
                                    mrow_sb[
                                        :, ih * NH + it8 * P : ih * NH + (it8 + 1) * P
                                    ],
                                    c_sb[:, h * VW : (h + 1) * VW],
                                    start=False,
                                    stop=(it8 % 4 == 3),
                                )
                            r_sb = sp.tile([P, NHT], f32, tag="r", name="r_sb")
                            pa3 = pa.rearrange("p (t c) -> p t c", t=NHT, c=P)
                            nc.vector.reciprocal(r_sb[:, :], pa3[:, :, DH])
                            for it8 in range(NHT):
                                it = ih * NHT + it8
                                dst = att_pair[
                                    :, it * P + p0 : it * P + p0 + DH
                                ]
                                nc.vector.tensor_scalar_mul(
                                    dst,
                                    pa[:, it8 * P : it8 * P + DH],
                                    r_sb[:, it8 : it8 + 1],
                                )
                            for ft_qu in next_chunks[2 * blk : 2 * blk + 2]:
                                emit_qk_chunk(*ft_qu)
                            blk += 1
                    for it in range(NT):
                        ps_tr = pqk.tile([P, P], bf, tag="qk", name="ps_tr")
                        nc.tensor.transpose(ps_tr[:, :], att_pair[:, ts(it, P)], ident)
                        nc.vector.tensor_copy(
                            attT[:, pair * N + it * P : pair * N + (it + 1) * P],
                            ps_tr[:, :],
                        )

                # ---- partial output projection ----
                # [128, 512] chunks so the tiles fit the 1-bank "qk" slots.
                # PSUM->SBUF eviction on the Scalar engine (idle here).
                COPY = mybir.ActivationFunctionType.Copy
                for it in range(NT):
                    for ch in range(2):
                        ps_o = pqk.tile([P, 512], f32, tag="qk", name="ps_o")
                        for ht in range(HT):
                            nc.tensor.matmul(
                                ps_o[:, :],
                                attT[:, ht * N + it * P : ht * N + (it + 1) * P],
                                wout_sb[
                                    :, ht * DM + ch * 512 : ht * DM + (ch + 1) * 512
                                ],
                                start=(ht == 0),
                                stop=(ht == HT - 1),
                            )
                        o_sb = sp.tile([P, 512], f32, tag="ob", name="o_sb")
                        # Alternate eviction engine so ACT and DVE each
                        # drain half the projection chunks in parallel.
                        if ch == 0:
                            nc.scalar.activation(o_sb[:, :], ps_o[:, :], COPY)
                        else:
                            nc.vector.tensor_copy(o_sb[:, :], ps_o[:, :])
                        nc.sync.dma_start(
                            out=out[ts(it, P), ts(ch, 512)], in_=o_sb[:, :]
                        )

    nc.compile()
    return nc


def _shard_inputs(x, w_qkv, w_out, b_out, mask):
    """Build the per-core input maps (host-side sharding + layout prep)."""
    bf = ml_dtypes.bfloat16
    x = np.asarray(x, dtype=np.float32)
    w_qkv = np.asarray(w_qkv, dtype=np.float32)
    w_out = np.asarray(w_out, dtype=np.float32)
    mask = np.asarray(mask)

    # w_qkv columns: head h occupies cols [h*192, (h+1)*192) as q|k|v of 64.
    w3 = w_qkv.reshape(DM, H, 3, DH)
    in_maps = []
    for c in range(NCORES):
        b, hg = c // HG, c % HG
        # q features for all 8 heads (cols 0:512, pre-scaled by SCALE),
        # then k features
        wqk_c = np.ascontiguousarray(
            np.concatenate(
                [
                    w3[:, hg * HL : (hg + 1) * HL, 0, :].reshape(DM, FV) * SCALE,
                    w3[:, hg * HL : (hg + 1) * HL, 1, :].reshape(DM, FV),
                ],
                axis=1,
            )
        ).astype(bf)
        wv_c = np.ascontiguousarray(
            w3[:, hg * HL : (hg + 1) * HL, 2, :].reshape(DM, FV)
        ).astype(bf)
        wout_c = np.ascontiguousarray(w_out[hg * FV : (hg + 1) * FV, :]).astype(bf)
        xT_c = np.ascontiguousarray(x[b].T).astype(bf)

        m = mask[b].astype(np.float32)  # [N] of 0/1
        qkm = np.broadcast_to(m[None, :], (P, N)).copy()
        mrow_c = m[None, :].astype(bf)
        iminv_c = np.ascontiguousarray((1.0 - m).reshape(NT, P).T).astype(bf)

        in_maps.append(
            {
                "xT": xT_c,
                "wqk": wqk_c,
                "wv": wv_c,
                "wout": wout_c,
                "qkmask": qkm,
                "mrow": mrow_c,
                "iminv": iminv_c,
            }
        )
    return in_maps


def kernel(x, w_qkv, w_out, b_out, mask):
    from concourse.bass_utils import run_bass_kernel_spmd

    if "nc" not in _CACHE:
        _CACHE["nc"] = _build_program()
    nc = _CACHE["nc"]

    in_maps = _shard_inputs(x, w_qkv, w_out, b_out, mask)
    res = run_bass_kernel_spmd(nc, in_maps, list(range(NCORES))).results

    b_out = np.asarray(b_out, dtype=np.float32)
    out = np.empty((B, N, DM), np.float32)
    for b in range(B):
        out[b] = res[HG * b]["out"] + res[HG * b + 1]["out"] + b_out[None, :]
    return out


# revision 30
# speedup vs baseline: 1.3198x; 1.0002x over previous
"""Multi-head attention (B=4, N=2048, DM=1024, H=16, DH=64) on 8 trn2 cores.

Sharding: core c -> (batch b = c//2, head-group hg = c%2 of 8 heads).
Each core computes qkv for its 8 heads, masked softmax attention, and a
partial output projection over its 512 head-dims.  Host sums the two
partials per batch and adds the bias.

Device-side layout ("feature-major"):
  - x^T [DM, N] so QK projection emits q^T/k^T [64, N] per head directly.
  - mask folded into q^T (x SCALE*m_i, SCALE pre-baked in w_q) and k^T
    (x m_j): masked score pairs become exp(0)=1; a rank-1 correction
    matmul (-m_i * C_h, with C_h = sum_{masked j} v_h[j,:]) cancels them
    for live queries, and dead queries (m_i=0) fall out as the exact
    uniform-softmax rows the reference produces.
  - v stored token-major with an appended ones column per head, so the
    PV matmul accumulates the softmax denominator for free.
  - per-head-pair QK projection is interleaved into the head loop so the
    PE keeps busy while ACT (exp) is the softmax bottleneck.
"""

import sys

sys.path.insert(0, "/opt/trn_rl_repo")

import numpy as np
import ml_dtypes

B, N, DM, H, DH = 4, 2048, 1024, 16, 64
SCALE = DH**-0.5
NCORES = 8
HG = 2  # head groups (tensor-parallel factor)
HL = H // HG  # 8 heads per core
NP = HL // 2  # 4 head pairs
FQK = HL * 2 * DH  # 1024 qk features per core
FV = HL * DH  # 512 v features per core
P = 128
NT = N // P  # 16 token tiles
DMT = DM // P  # 8 dm tiles
VW = DH + 1  # 65: v columns + ones column
VROW = HL * VW  # 520
HT = FV // P  # 4 head-dim tiles for the projection
NH = N // 2  # 1024: i-half width
NHT = NT // 2  # 8 token tiles per i-half

_CACHE = {}


def _build_program():
    import concourse.mybir as mybir
    import concourse.tile as tile
    from concourse import bacc
    from concourse.bass import ts
    from concourse.masks import make_identity

    bf = mybir.dt.bfloat16
    f32 = mybir.dt.float32
    EXP = mybir.ActivationFunctionType.Exp

    nc = bacc.Bacc(
        "TRN2", target_bir_lowering=False, debug=False, num_devices=NCORES
    )
    xT = nc.dram_tensor("xT", [DM, N], bf, kind="ExternalInput")
    wqk = nc.dram_tensor("wqk", [DM, FQK], bf, kind="ExternalInput")
    wv = nc.dram_tensor("wv", [DM, FV], bf, kind="ExternalInput")
    wout = nc.dram_tensor("wout", [FV, DM], bf, kind="ExternalInput")
    qkmask = nc.dram_tensor("qkmask", [P, N], f32, kind="ExternalInput")
    mrow = nc.dram_tensor("mrow", [1, N], bf, kind="ExternalInput")
    iminv = nc.dram_tensor("iminv", [P, NT], bf, kind="ExternalInput")
    out = nc.dram_tensor("out", [N, DM], f32, kind="ExternalOutput")

    with tile.TileContext(nc) as tc:
        with tc.tile_pool(name="const", bufs=1) as cp:
            xT_sb = cp.tile([P, DMT * N], bf, tag="xT")
            wqk_sb = cp.tile([P, DMT * FQK], bf, tag="wqk")
            wv_sb = cp.tile([P, DMT * FV], bf, tag="wv")
            wout_sb = cp.tile([P, HT * DM], bf, tag="wout")
            qkm_sb = cp.tile([P, N], f32, tag="qkm")
            mrow_sb = cp.tile([1, N], bf, tag="mrow")
            iminv_sb = cp.tile([P, NT], bf, tag="iminv")
            ident = cp.tile([P, P], bf, tag="ident")
            vplus = cp.tile([P, NT * VROW], bf, tag="vplus")
            qk_all = cp.tile([P, HL * N], bf, tag="qkall")
            attT = cp.tile([P, HT * N], bf, tag="attT")
            att_pair = cp.tile([P, NT * P], bf, tag="attpair")
            c_sb = cp.tile([1, VROW], bf, tag="csb")

            # DMA order mirrors consumption: the first QK chunk's inputs
            # (xT/wqk dmt 0) lead, then the small mask tensors (the DVE
            # program's first op waits on qkmask), then the remaining
            # xT/wqk tiles; wv and wout are needed later so they go last.
            nc.sync.dma_start(out=xT_sb[:, ts(0, N)], in_=xT[ts(0, P), :])
            nc.sync.dma_start(out=wqk_sb[:, ts(0, FQK)], in_=wqk[ts(0, P), :])
            nc.sync.dma_start(out=qkm_sb[:, :], in_=qkmask[:, :])
            nc.sync.dma_start(out=mrow_sb[:, :], in_=mrow[:, :])
            nc.sync.dma_start(out=iminv_sb[:, :], in_=iminv[:, :])
            for dmt in range(1, DMT):
                nc.sync.dma_start(out=xT_sb[:, ts(dmt, N)], in_=xT[ts(dmt, P), :])
                nc.sync.dma_start(out=wqk_sb[:, ts(dmt, FQK)], in_=wqk[ts(dmt, P), :])
            for dmt in range(DMT):
                nc.sync.dma_start(out=wv_sb[:, ts(dmt, FV)], in_=wv[ts(dmt, P), :])
            for ht in range(HT):
                nc.sync.dma_start(out=wout_sb[:, ts(ht, DM)], in_=wout[ts(ht, P), :])
            make_identity(nc, ident)

            vp4 = vplus.rearrange("p (t g c) -> p t g c", t=NT, g=HL, c=VW)
            nc.gpsimd.memset(vp4[:, :, :, DH], 1.0)

            # Prime the DVE vector clock on the mask DMA so the first
            # tensor_mul needs only the PE wait.
            scratch = cp.tile([1, 1], f32, tag="scratch")
            nc.vector.tensor_copy(scratch, qkm_sb[0:1, 0:1])

            with (
                tc.tile_pool(name="psqk", bufs=2, space="PSUM") as pqk,
                tc.tile_pool(name="pss", bufs=2, space="PSUM") as pss,
                tc.tile_pool(name="psa", bufs=1, space="PSUM") as psa,
                tc.tile_pool(name="tpool", bufs=27) as tp,
                tc.tile_pool(name="spool", bufs=4) as sp,
            ):

                def emit_qk_chunk(ft, qu):
                    ps_qk = pqk.tile([P, 512], f32, tag="qk", name="ps_qk")
                    for dmt in range(DMT):
                        nc.tensor.matmul(
                            ps_qk[:, :],
                            wqk_sb[:, dmt * FQK + ft * P : dmt * FQK + (ft + 1) * P],
                            xT_sb[:, dmt * N + qu * 512 : dmt * N + (qu + 1) * 512],
                            start=(dmt == 0),
                            stop=(dmt == DMT - 1),
                        )
                    nc.vector.tensor_mul(
                        qk_all[:, ft * N + qu * 512 : ft * N + (qu + 1) * 512],
                        ps_qk[:, :],
                        qkm_sb[:, ts(qu, 512)],
                    )

                def emit_qk(pair):
                    # q f-tile `pair` and k f-tile NP+pair, in N-quarters.
                    for ft in (pair, NP + pair):
                        for qu in range(4):
                            emit_qk_chunk(ft, qu)

                emit_qk(0)

                # Pre-emit head 0 / i-half 0 scores+exp ahead of the V
                # projection so ACT starts working ~16us in instead of
                # waiting for V+C (~60us).  The t tiles are consumed by the
                # regular PV loop below once vplus is ready.
                pre_t = []
                for jt in range(NT):
                    t_sb = tp.tile([P, NH], bf, tag="t", name="t_sb")
                    kT = qk_all[0:64, NP * N + jt * P : NP * N + (jt + 1) * P]
                    ps_s = pss.tile([P, NH], f32, tag="s", name="ps_s")
                    for ch in range(2):
                        nc.tensor.matmul(
                            ps_s[:, ts(ch, 512)],
                            kT,
                            qk_all[0:64, ch * 512 : (ch + 1) * 512],
                            start=True,
                            stop=True,
                        )
                    nc.scalar.activation(t_sb[:, :], ps_s[:, :], EXP)
                    pre_t.append(t_sb)

                # ---- V projection (token-major) + masked-v row C ----
                for tt in range(NT):
                    ps_v = pqk.tile([P, FV], f32, tag="qk", name="ps_v")
                    for dmt in range(DMT):
                        nc.tensor.matmul(
                            ps_v[:, :],
                            xT_sb[:, dmt * N + tt * P : dmt * N + (tt + 1) * P],
                            wv_sb[:, ts(dmt, FV)],
                            start=(dmt == 0),
                            stop=(dmt == DMT - 1),
                        )
                    nc.vector.tensor_copy(
                        vp4[:, tt, :, 0:DH],
                        ps_v.rearrange("p (g c) -> p g c", g=HL, c=DH),
                    )
                # C tiles live in the 1-bank "qk" slots: putting them in the
                # scores pool would pin both scores slots until the whole V
                # projection finishes, stalling ACT ~76us at startup.
                ps_c0 = pqk.tile([1, VROW // 2], f32, tag="qk", name="ps_c0")
                ps_c1 = pqk.tile([1, VROW // 2], f32, tag="qk", name="ps_c1")
                for jt in range(NT):
                    nc.tensor.matmul(
                        ps_c0[:, :],
                        iminv_sb[:, jt : jt + 1],
                        vplus[:, jt * VROW : jt * VROW + VROW // 2],
                        start=(jt == 0),
                        stop=(jt == NT - 1),
                    )
                    nc.tensor.matmul(
                        ps_c1[:, :],
                        iminv_sb[:, jt : jt + 1],
                        vplus[:, jt * VROW + VROW // 2 : (jt + 1) * VROW],
                        start=(jt == 0),
                        stop=(jt == NT - 1),
                    )
                nc.vector.tensor_scalar_mul(c_sb[:, 0 : VROW // 2], ps_c0[:, :], -1.0)
                nc.vector.tensor_scalar_mul(
                    c_sb[:, VROW // 2 : VROW], ps_c1[:, :], -1.0
                )

                # ---- head loop: scores^T -> exp -> PV -> normalize ----
                # Next pair's QK projection is spread 2 chunks per
                # (head, i-half) block so the PE fills its slack inside the
                # ACT-bound softmax phase instead of stalling ACT at pair
                # boundaries.
                for pair in range(NP):
                    next_chunks = (
                        [(ft, qu) for ft in (pair + 1, NP + pair + 1) for qu in range(4)]
                        if pair + 1 < NP
                        else []
                    )
                    blk = 0
                    for hh in range(2):
                        h = 2 * pair + hh
                        p0 = hh * 64
                        qcol = pair * N
                        kcol = (NP + pair) * N
                        for ih in range(2):
                            pa = psa.tile([P, NH], f32, tag="att", name="pa")
                            for jt in range(NT):
                                if h == 0 and ih == 0:
                                    t_sb = pre_t[jt]
                                else:
                                    t_sb = tp.tile([P, NH], bf, tag="t", name="t_sb")
                                    kT = qk_all[
                                        p0 : p0 + 64,
                                        kcol + jt * P : kcol + (jt + 1) * P,
                                    ]
                                    ps_s = pss.tile([P, NH], f32, tag="s", name="ps_s")
                                    for ch in range(2):
                                        c0 = qcol + ih * NH + ch * 512
                                        nc.tensor.matmul(
                                            ps_s[:, ts(ch, 512)],
                                            kT,
                                            qk_all[p0 : p0 + 64, c0 : c0 + 512],
                                            start=True,
                                            stop=True,
                                        )
                                    nc.scalar.activation(t_sb[:, :], ps_s[:, :], EXP)
                                vslice = vplus[
                                    :, jt * VROW + h * VW : jt * VROW + (h + 1) * VW
                                ]
                                for it8 in range(NHT):
                                    nc.tensor.matmul(
                                        pa[:, it8 * P : it8 * P + VW],
                                        t_sb[:, ts(it8, P)],
                                        vslice,
                                        start=(jt == 0 and it8 % 4 == 0),
                                        stop=False,
                                    )
                            for it8 in range(NHT):
                                nc.tensor.matmul(
                                    pa[:, it8 * P : it8 * P + VW],
                                    mrow_sb[
                                        :, ih * NH + it8 * P : ih * NH + (it8 + 1) * P
                                    ],
                                    c_sb[:, h * VW : (h + 1) * VW],
                                    start=False,
                                    stop=(it8 % 4 == 3),
                                )
                            r_sb = sp.tile([P, NHT], f32, tag="r", name="r_sb")
                            pa3 = pa.rearrange("p (t c) -> p t c", t=NHT, c=P)
                            nc.vector.reciprocal(r_sb[:, :], pa3[:, :, DH])
                            for it8 in range(NHT):
                                it = ih * NHT + it8
                                dst = att_pair[
                                    :, it * P + p0 : it * P + p0 + DH
                                ]
                                nc.vector.tensor_scalar_mul(
                                    dst,
                                    pa[:, it8 * P : it8 * P + DH],
                                    r_sb[:, it8 : it8 + 1],
                                )
                            for ft_qu in next_chunks[2 * blk : 2 * blk + 2]:
                                emit_qk_chunk(*ft_qu)
                            blk += 1
                    for it in range(NT):
                        ps_tr = pqk.tile([P, P], bf, tag="qk", name="ps_tr")
                        nc.tensor.transpose(ps_tr[:, :], att_pair[:, ts(it, P)], ident)
                        nc.vector.tensor_copy(
                            attT[:, pair * N + it * P : pair * N + (it + 1) * P],
                            ps_tr[:, :],
                        )

                # ---- partial output projection ----
                # [128, 512] chunks so the tiles fit the 1-bank "qk" slots.
                # PSUM->SBUF eviction on the Scalar engine (idle here).
                COPY = mybir.ActivationFunctionType.Copy
                for it in range(NT):
                    for ch in range(2):
                        ps_o = pqk.tile([P, 512], f32, tag="qk", name="ps_o")
                        for ht in range(HT):
                            nc.tensor.matmul(
                                ps_o[:, :],
                                attT[:, ht * N + it * P : ht * N + (it + 1) * P],
                                wout_sb[
                                    :, ht * DM + ch * 512 : ht * DM + (ch + 1) * 512
                                ],
                                start=(ht == 0),
                                stop=(ht == HT - 1),
                            )
                        o_sb = sp.tile([P, 512], f32, tag="ob", name="o_sb")
                        # Alternate eviction engine so ACT and DVE each
                        # drain half the projection chunks in parallel.
                        if ch == 0:
                            nc.scalar.activation(o_sb[:, :], ps_o[:, :], COPY)
                        else:
                            nc.vector.tensor_copy(o_sb[:, :], ps_o[:, :])
                        nc.sync.dma_start(
                            out=out[ts(it, P), ts(ch, 512)], in_=o_sb[:, :]
                        )

    nc.compile()
    return nc


def _shard_inputs(x, w_qkv, w_out, b_out, mask):
    """Build the per-core input maps (host-side sharding + layout prep)."""
    bf = ml_dtypes.bfloat16
    x = np.asarray(x, dtype=np.float32)
    w_qkv = np.asarray(w_qkv, dtype=np.float32)
    w_out = np.asarray(w_out, dtype=np.float32)
    mask = np.asarray(mask)

    # w_qkv columns: head h occupies cols [h*192, (h+1)*192) as q|k|v of 64.
    w3 = w_qkv.reshape(DM, H, 3, DH)
    in_maps = []
    for c in range(NCORES):
        b, hg = c // HG, c % HG
        # q features for all 8 heads (cols 0:512, pre-scaled by SCALE),
        # then k features
        wqk_c = np.ascontiguousarray(
            np.concatenate(
                [
                    w3[:, hg * HL : (hg + 1) * HL, 0, :].reshape(DM, FV) * SCALE,
                    w3[:, hg * HL : (hg + 1) * HL, 1, :].reshape(DM, FV),
                ],
                axis=1,
            )
        ).astype(bf)
        wv_c = np.ascontiguousarray(
            w3[:, hg * HL : (hg + 1) * HL, 2, :].reshape(DM, FV)
        ).astype(bf)
        wout_c = np.ascontiguousarray(w_out[hg * FV : (hg + 1) * FV, :]).astype(bf)
        xT_c = np.ascontiguousarray(x[b].T).astype(bf)

        m = mask[b].astype(np.float32)  # [N] of 0/1
        qkm = np.broadcast_to(m[None, :], (P, N)).copy()
        mrow_c = m[None, :].astype(bf)
        iminv_c = np.ascontiguousarray((1.0 - m).reshape(NT, P).T).astype(bf)

        in_maps.append(
            {
                "xT": xT_c,
                "wqk": wqk_c,
                "wv": wv_c,
                "wout": wout_c,
                "qkmask": qkm,
                "mrow": mrow_c,
                "iminv": iminv_c,
            }
        )
    return in_maps


def kernel(x, w_qkv, w_out, b_out, mask):
    from concourse.bass_utils import run_bass_kernel_spmd

    if "nc" not in _CACHE:
        _CACHE["nc"] = _build_program()
    nc = _CACHE["nc"]

    in_maps = _shard_inputs(x, w_qkv, w_out, b_out, mask)
    res = run_bass_kernel_spmd(nc, in_maps, list(range(NCORES))).results

    b_out = np.asarray(b_out, dtype=np.float32)
    out = np.empty((B, N, DM), np.float32)
    for b in range(B):
        out[b] = res[HG * b]["out"] + res[HG * b + 1]["out"] + b_out[None, :]
    return out


# revision 32
# speedup vs baseline: 1.3286x; 1.0067x over previous
"""Multi-head attention (B=4, N=2048, DM=1024, H=16, DH=64) on 8 trn2 cores.

Sharding: core c -> (batch b = c//2, head-group hg = c%2 of 8 heads).
Each core computes qkv for its 8 heads, masked softmax attention, and a
partial output projection over its 512 head-dims.  Host sums the two
partials per batch and adds the bias.

Device-side layout ("feature-major"):
  - x^T [DM, N] so QK projection emits q^T/k^T [64, N] per head directly.
  - mask folded into q^T (x SCALE*m_i, SCALE pre-baked in w_q) and k^T
    (x m_j): masked score pairs become exp(0)=1; a rank-1 correction
    matmul (-m_i * C_h, with C_h = sum_{masked j} v_h[j,:]) cancels them
    for live queries, and dead queries (m_i=0) fall out as the exact
    uniform-softmax rows the reference produces.
  - v stored token-major with an appended ones column per head, so the
    PV matmul accumulates the softmax denominator for free.
  - per-head-pair QK projection is interleaved into the head loop so the
    PE keeps busy while ACT (exp) is the softmax bottleneck.
"""

import sys

sys.path.insert(0, "/opt/trn_rl_repo")

import numpy as np
import ml_dtypes

B, N, DM, H, DH = 4, 2048, 1024, 16, 64
SCALE = DH**-0.5
NCORES = 8
HG = 2  # head groups (tensor-parallel factor)
HL = H // HG  # 8 heads per core
NP = HL // 2  # 4 head pairs
FQK = HL * 2 * DH  # 1024 qk features per core
FV = HL * DH  # 512 v features per core
P = 128
NT = N // P  # 16 token tiles
DMT = DM // P  # 8 dm tiles
VW = DH + 1  # 65: v columns + ones column
VROW = HL * VW  # 520
HT = FV // P  # 4 head-dim tiles for the projection
NH = N // 2  # 1024: i-half width
NHT = NT // 2  # 8 token tiles per i-half

_CACHE = {}


def _build_program():
    import concourse.mybir as mybir
    import concourse.tile as tile
    from concourse import bacc
    from concourse.bass import ts
    from concourse.masks import make_identity

    bf = mybir.dt.bfloat16
    f32 = mybir.dt.float32
    EXP = mybir.ActivationFunctionType.Exp

    nc = bacc.Bacc(
        "TRN2", target_bir_lowering=False, debug=False, num_devices=NCORES
    )
    xT = nc.dram_tensor("xT", [DM, N], bf, kind="ExternalInput")
    wqk = nc.dram_tensor("wqk", [DM, FQK], bf, kind="ExternalInput")
    wv = nc.dram_tensor("wv", [DM, FV], bf, kind="ExternalInput")
    wout = nc.dram_tensor("wout", [FV, DM], bf, kind="ExternalInput")
    qkmask = nc.dram_tensor("qkmask", [P, N], f32, kind="ExternalInput")
    mrow = nc.dram_tensor("mrow", [1, N], bf, kind="ExternalInput")
    iminv = nc.dram_tensor("iminv", [P, NT], bf, kind="ExternalInput")
    out = nc.dram_tensor("out", [N, DM], f32, kind="ExternalOutput")

    with tile.TileContext(nc) as tc:
        with tc.tile_pool(name="const", bufs=1) as cp:
            xT_sb = cp.tile([P, DMT * N], bf, tag="xT")
            wqk_sb = cp.tile([P, DMT * FQK], bf, tag="wqk")
            wv_sb = cp.tile([P, DMT * FV], bf, tag="wv")
            wout_sb = cp.tile([P, HT * DM], bf, tag="wout")
            qkm_sb = cp.tile([P, N], f32, tag="qkm")
            mrow_sb = cp.tile([1, N], bf, tag="mrow")
            iminv_sb = cp.tile([P, NT], bf, tag="iminv")
            ident = cp.tile([P, P], bf, tag="ident")
            vplus = cp.tile([P, NT * VROW], bf, tag="vplus")
            qk_all = cp.tile([P, HL * N], bf, tag="qkall")
            attT = cp.tile([P, HT * N], bf, tag="attT")
            att_pair = cp.tile([P, NT * P], bf, tag="attpair")
            c_sb = cp.tile([1, VROW], bf, tag="csb")

            # DMA order mirrors consumption: the first QK chunk's inputs
            # (xT/wqk dmt 0) lead, then the small mask tensors (the DVE
            # program's first op waits on qkmask), then the remaining
            # xT/wqk tiles; wv and wout are needed later so they go last.
            nc.sync.dma_start(out=xT_sb[:, ts(0, N)], in_=xT[ts(0, P), :])
            nc.sync.dma_start(out=wqk_sb[:, ts(0, FQK)], in_=wqk[ts(0, P), :])
            nc.sync.dma_start(out=qkm_sb[:, :], in_=qkmask[:, :])
            nc.sync.dma_start(out=mrow_sb[:, :], in_=mrow[:, :])
            nc.sync.dma_start(out=iminv_sb[:, :], in_=iminv[:, :])
            for dmt in range(1, DMT):
                nc.sync.dma_start(out=xT_sb[:, ts(dmt, N)], in_=xT[ts(dmt, P), :])
                nc.sync.dma_start(out=wqk_sb[:, ts(dmt, FQK)], in_=wqk[ts(dmt, P), :])
            for dmt in range(DMT):
                nc.sync.dma_start(out=wv_sb[:, ts(dmt, FV)], in_=wv[ts(dmt, P), :])
            for ht in range(HT):
                nc.sync.dma_start(out=wout_sb[:, ts(ht, DM)], in_=wout[ts(ht, P), :])
            make_identity(nc, ident)

            vp4 = vplus.rearrange("p (t g c) -> p t g c", t=NT, g=HL, c=VW)
            nc.gpsimd.memset(vp4[:, :, :, DH], 1.0)

            # Prime the DVE vector clock on the mask DMA so the first
            # tensor_mul needs only the PE wait.
            scratch = cp.tile([1, 1], f32, tag="scratch")
            nc.vector.tensor_copy(scratch, qkm_sb[0:1, 0:1])

            with (
                tc.tile_pool(name="psqk", bufs=2, space="PSUM") as pqk,
                tc.tile_pool(name="pss", bufs=2, space="PSUM") as pss,
                tc.tile_pool(name="psa", bufs=1, space="PSUM") as psa,
                tc.tile_pool(name="tpool", bufs=27) as tp,
                tc.tile_pool(name="spool", bufs=4) as sp,
            ):

                def emit_qk_chunk(ft, qu):
                    ps_qk = pqk.tile([P, 512], f32, tag="qk", name="ps_qk")
                    for dmt in range(DMT):
                        nc.tensor.matmul(
                            ps_qk[:, :],
                            wqk_sb[:, dmt * FQK + ft * P : dmt * FQK + (ft + 1) * P],
                            xT_sb[:, dmt * N + qu * 512 : dmt * N + (qu + 1) * 512],
                            start=(dmt == 0),
                            stop=(dmt == DMT - 1),
                        )
                    nc.vector.tensor_mul(
                        qk_all[:, ft * N + qu * 512 : ft * N + (qu + 1) * 512],
                        ps_qk[:, :],
                        qkm_sb[:, ts(qu, 512)],
                    )

                def emit_qk(pair):
                    # q f-tile `pair` and k f-tile NP+pair, in N-quarters.
                    for ft in (pair, NP + pair):
                        for qu in range(4):
                            emit_qk_chunk(ft, qu)

                emit_qk(0)

                # Pre-emit head 0 / i-half 0 scores+exp ahead of the V
                # projection so ACT starts working ~16us in instead of
                # waiting for V+C (~60us).  The t tiles are consumed by the
                # regular PV loop below once vplus is ready.
                pre_t = []
                for ih in range(2):
                    for jt in range(NT):
                        t_sb = tp.tile([P, NH], bf, tag="t", name="t_sb")
                        kT = qk_all[0:64, NP * N + jt * P : NP * N + (jt + 1) * P]
                        ps_s = pss.tile([P, NH], f32, tag="s", name="ps_s")
                        for ch in range(2):
                            c0 = ih * NH + ch * 512
                            nc.tensor.matmul(
                                ps_s[:, ts(ch, 512)],
                                kT,
                                qk_all[0:64, c0 : c0 + 512],
                                start=True,
                                stop=True,
                            )
                        nc.scalar.activation(t_sb[:, :], ps_s[:, :], EXP)
                        pre_t.append(t_sb)

                # ---- V projection (token-major) + masked-v row C ----
                for tt in range(NT):
                    ps_v = pqk.tile([P, FV], f32, tag="qk", name="ps_v")
                    for dmt in range(DMT):
                        nc.tensor.matmul(
                            ps_v[:, :],
                            xT_sb[:, dmt * N + tt * P : dmt * N + (tt + 1) * P],
                            wv_sb[:, ts(dmt, FV)],
                            start=(dmt == 0),
                            stop=(dmt == DMT - 1),
                        )
                    nc.vector.tensor_copy(
                        vp4[:, tt, :, 0:DH],
                        ps_v.rearrange("p (g c) -> p g c", g=HL, c=DH),
                    )
                # C tiles live in the 1-bank "qk" slots: putting them in the
                # scores pool would pin both scores slots until the whole V
                # projection finishes, stalling ACT ~76us at startup.
                ps_c0 = pqk.tile([1, VROW // 2], f32, tag="qk", name="ps_c0")
                ps_c1 = pqk.tile([1, VROW // 2], f32, tag="qk", name="ps_c1")
                for jt in range(NT):
                    nc.tensor.matmul(
                        ps_c0[:, :],
                        iminv_sb[:, jt : jt + 1],
                        vplus[:, jt * VROW : jt * VROW + VROW // 2],
                        start=(jt == 0),
                        stop=(jt == NT - 1),
                    )
                    nc.tensor.matmul(
                        ps_c1[:, :],
                        iminv_sb[:, jt : jt + 1],
                        vplus[:, jt * VROW + VROW // 2 : (jt + 1) * VROW],
                        start=(jt == 0),
                        stop=(jt == NT - 1),
                    )
                nc.vector.tensor_scalar_mul(c_sb[:, 0 : VROW // 2], ps_c0[:, :], -1.0)
                nc.vector.tensor_scalar_mul(
                    c_sb[:, VROW // 2 : VROW], ps_c1[:, :], -1.0
                )

                # ---- head loop: scores^T -> exp -> PV -> normalize ----
                # Next pair's QK projection is spread 2 chunks per
                # (head, i-half) block so the PE fills its slack inside the
                # ACT-bound softmax phase instead of stalling ACT at pair
                # boundaries.
                for pair in range(NP):
                    next_chunks = (
                        [(ft, qu) for ft in (pair + 1, NP + pair + 1) for qu in range(4)]
                        if pair + 1 < NP
                        else []
                    )
                    blk = 0
                    for hh in range(2):
                        h = 2 * pair + hh
                        p0 = hh * 64
                        qcol = pair * N
                        kcol = (NP + pair) * N
                        for ih in range(2):
                            pa = psa.tile([P, NH], f32, tag="att", name="pa")
                            for jt in range(NT):
                                if h == 0:
                                    t_sb = pre_t[ih * NT + jt]
                                else:
                                    t_sb = tp.tile([P, NH], bf, tag="t", name="t_sb")
                                    kT = qk_all[
                                        p0 : p0 + 64,
                                        kcol + jt * P : kcol + (jt + 1) * P,
                                    ]
                                    ps_s = pss.tile([P, NH], f32, tag="s", name="ps_s")
                                    for ch in range(2):
                                        c0 = qcol + ih * NH + ch * 512
                                        nc.tensor.matmul(
                                            ps_s[:, ts(ch, 512)],
                                            kT,
                                            qk_all[p0 : p0 + 64, c0 : c0 + 512],
                                            start=True,
                                            stop=True,
                                        )
                                    nc.scalar.activation(t_sb[:, :], ps_s[:, :], EXP)
                                vslice = vplus[
                                    :, jt * VROW + h * VW : jt * VROW + (h + 1) * VW
                                ]
                                for it8 in range(NHT):
                                    nc.tensor.matmul(
                                        pa[:, it8 * P : it8 * P + VW],
                                        t_sb[:, ts(it8, P)],
                                        vslice,
                                        start=(jt == 0 and it8 % 4 == 0),
                                        stop=False,
                                    )
                            for it8 in range(NHT):
                                nc.tensor.matmul(
                                    pa[:, it8 * P : it8 * P + VW],
                                    mrow_sb[
                                        :, ih * NH + it8 * P : ih * NH + (it8 + 1) * P
                                    ],
                                    c_sb[:, h * VW : (h + 1) * VW],
                                    start=False,
                                    stop=(it8 % 4 == 3),
                                )
                            r_sb = sp.tile([P, NHT], f32, tag="r", name="r_sb")
                            pa3 = pa.rearrange("p (t c) -> p t c", t=NHT, c=P)
                            nc.vector.reciprocal(r_sb[:, :], pa3[:, :, DH])
                            for it8 in range(NHT):
                                it = ih * NHT + it8
                                dst = att_pair[
                                    :, it * P + p0 : it * P + p0 + DH
                                ]
                                nc.vector.tensor_scalar_mul(
                                    dst,
                                    pa[:, it8 * P : it8 * P + DH],
                                    r_sb[:, it8 : it8 + 1],
                                )
                            for ft_qu in next_chunks[2 * blk : 2 * blk + 2]:
                                emit_qk_chunk(*ft_qu)
                            blk += 1
                    for it in range(NT):
                        ps_tr = pqk.tile([P, P], bf, tag="qk", name="ps_tr")
                        nc.tensor.transpose(ps_tr[:, :], att_pair[:, ts(it, P)], ident)
                        nc.vector.tensor_copy(
                            attT[:, pair * N + it * P : pair * N + (it + 1) * P],
                            ps_tr[:, :],
                        )

                # ---- partial output projection ----
                # [128, 512] chunks so the tiles fit the 1-bank "qk" slots.
                # PSUM->SBUF eviction on the Scalar engine (idle here).
                COPY = mybir.ActivationFunctionType.Copy
                for it in range(NT):
                    for ch in range(2):
                        ps_o = pqk.tile([P, 512], f32, tag="qk", name="ps_o")
                        for ht in range(HT):
                            nc.tensor.matmul(
                                ps_o[:, :],
                                attT[:, ht * N + it * P : ht * N + (it + 1) * P],
                                wout_sb[
                                    :, ht * DM + ch * 512 : ht * DM + (ch + 1) * 512
                                ],
                                start=(ht == 0),
                                stop=(ht == HT - 1),
                            )
                        o_sb = sp.tile([P, 512], f32, tag="ob", name="o_sb")
                        # Alternate eviction engine so ACT and DVE each
                        # drain half the projection chunks in parallel.
                        if ch == 0:
                            nc.scalar.activation(o_sb[:, :], ps_o[:, :], COPY)
                        else:
                            nc.vector.tensor_copy(o_sb[:, :], ps_o[:, :])
                        nc.sync.dma_start(
                            out=out[ts(it, P), ts(ch, 512)], in_=o_sb[:, :]
                        )

    nc.compile()
    return nc


def _shard_inputs(x, w_qkv, w_out, b_out, mask):
    """Build the per-core input maps (host-side sharding + layout prep)."""
    bf = ml_dtypes.bfloat16
    x = np.asarray(x, dtype=np.float32)
    w_qkv = np.asarray(w_qkv, dtype=np.float32)
    w_out = np.asarray(w_out, dtype=np.float32)
    mask = np.asarray(mask)

    # w_qkv columns: head h occupies cols [h*192, (h+1)*192) as q|k|v of 64.
    w3 = w_qkv.reshape(DM, H, 3, DH)
    in_maps = []
    for c in range(NCORES):
        b, hg = c // HG, c % HG
        # q features for all 8 heads (cols 0:512, pre-scaled by SCALE),
        # then k features
        wqk_c = np.ascontiguousarray(
            np.concatenate(
                [
                    w3[:, hg * HL : (hg + 1) * HL, 0, :].reshape(DM, FV) * SCALE,
                    w3[:, hg * HL : (hg + 1) * HL, 1, :].reshape(DM, FV),
                ],
                axis=1,
            )
        ).astype(bf)
        wv_c = np.ascontiguousarray(
            w3[:, hg * HL : (hg + 1) * HL, 2, :].reshape(DM, FV)
        ).astype(bf)
        wout_c = np.ascontiguousarray(w_out[hg * FV : (hg + 1) * FV, :]).astype(bf)
        xT_c = np.ascontiguousarray(x[b].T).astype(bf)

        m = mask[b].astype(np.float32)  # [N] of 0/1
        qkm = np.broadcast_to(m[None, :], (P, N)).copy()
        mrow_c = m[None, :].astype(bf)
        iminv_c = np.ascontiguousarray((1.0 - m).reshape(NT, P).T).astype(bf)

        in_maps.append(
            {
                "xT": xT_c,
                "wqk": wqk_c,
                "wv": wv_c,
                "wout": wout_c,
                "qkmask": qkm,
                "mrow": mrow_c,
                "iminv": iminv_c,
            }
        )
    return in_maps


def kernel(x, w_qkv, w_out, b_out, mask):
    from concourse.bass_utils import run_bass_kernel_spmd

    if "nc" not in _CACHE:
        _CACHE["nc"] = _build_program()
    nc = _CACHE["nc"]

    in_maps = _shard_inputs(x, w_qkv, w_out, b_out, mask)
    res = run_bass_kernel_spmd(nc, in_maps, list(range(NCORES))).results

    b_out = np.asarray(b_out, dtype=np.float32)
    out = np.empty((B, N, DM), np.float32)
    for b in range(B):
        out[b] = res[HG * b]["out"] + res[HG * b + 1]["out"] + b_out[None, :]
    return out
